# revision 9
# baseline (speedup 1.0000x reference)
"""Trainium2 Bass kernel for ClippingAttentionEngine.

Sharding: core c -> (batch b = c//2, head-group hg = c%2, 8 heads each).
Each core computes Q/K/V projections for its 8 heads, attention, and the
partial output projection over its head slice; host sums the two per-batch
partials (tensor-parallel over heads, per the sharding hint).

The per-batch sparse/dense branch is folded into a single dense-shaped
program via a MULTIPLICATIVE bias matrix M (bf16, transposed [k,q]):
dense batches get M = mask ? 1 : exp(-lambda); sparse batches get
M = multiplicity count of each key under prior_indices/prior_index_mask
(0 where never indexed). Then softmax(QK^T*scale + log M) == normalize
(exp(QK^T*scale) * M), so P = exp(scores) * M reproduces both branches
exactly. M is applied by the Vector engine (bf16 2x/4x mode) instead of
injecting log M into PSUM with identity matmuls, which frees ~128 tensor
engine instructions per core.

All matmul operands are bf16 (PSUM accumulation stays fp32). Input DMAs
are emitted in first-use order (xt/wq chunks interleaved) so the first
projection matmul can start ~5us in instead of ~27us. The output
projection for the first half of the queries is emitted in the middle of
stage B so its matmuls and DMAs overlap attention of the second half.
"""

import sys

sys.path.insert(0, "/opt/trn_rl_repo")

import ml_dtypes
import numpy as np

import concourse.bass as bass
import concourse.tile as tile
from concourse import bacc, mybir
from concourse.alu_op_type import AluOpType
from concourse.bass_utils import run_bass_kernel_spmd

B, S, D, H = 4, 1024, 1024, 16
DH = D // H          # 64
HPC = 8              # heads per core
N_CORES = 8
KT = S // 128        # 8 k tiles
DCH = D // 128       # 8 contraction chunks
LAMBDA_MAX, ALPHA, SPARSE_THRESHOLD = 10.0, 5.0, 1.0

F32 = mybir.dt.float32
BF16 = mybir.dt.bfloat16
EXP = mybir.ActivationFunctionType.Exp
IDENT = mybir.ActivationFunctionType.Identity


def build_program():
    nc = bacc.Bacc("TRN2", target_bir_lowering=False, debug=False,
                   num_devices=N_CORES)

    d_xt = nc.dram_tensor("xt", [D, S], BF16, kind="ExternalInput").ap()
    d_wqt = nc.dram_tensor("wqt", [D, 512], BF16, kind="ExternalInput").ap()
    d_wkt = nc.dram_tensor("wkt", [D, 512], BF16, kind="ExternalInput").ap()
    d_wvt = nc.dram_tensor("wvt", [D, 512], BF16, kind="ExternalInput").ap()
    d_wot = nc.dram_tensor("wot", [512, D], BF16, kind="ExternalInput").ap()
    d_bt = nc.dram_tensor("bt", [S, S], BF16, kind="ExternalInput").ap()
    d_bq = nc.dram_tensor("bq", [128, 4], F32, kind="ExternalInput").ap()
    d_bk = nc.dram_tensor("bk", [128, 4], F32, kind="ExternalInput").ap()
    d_bv = nc.dram_tensor("bv", [1, 512], BF16, kind="ExternalInput").ap()
    d_bo = nc.dram_tensor("bo", [1, D], BF16, kind="ExternalInput").ap()
    d_ones = nc.dram_tensor("ones", [128, 512], BF16, kind="ExternalInput").ap()
    d_out = nc.dram_tensor("out", [S, D], F32, kind="ExternalOutput").ap()

    with tile.TileContext(nc) as tc:
        with (
            tc.tile_pool(name="const", bufs=1) as constp,
            tc.tile_pool(name="main", bufs=1) as mainp,
        ):
            # Persistent arrays (bf16).
            qt_sb = [mainp.tile([128, S], BF16, tag=f"qt{m}", name=f"qt{m}")
                     for m in range(4)]
            kt_sb = [mainp.tile([128, S], BF16, tag=f"kt{m}", name=f"kt{m}")
                     for m in range(4)]
            vp_sb = [mainp.tile([128, HPC * (DH + 1)], BF16, tag=f"vp{sb}",
                                name=f"vp{sb}") for sb in range(8)]
            at_sb = [mainp.tile([128, S], BF16, tag=f"at{m}", name=f"at{m}")
                     for m in range(4)]
            wot_sb = [mainp.tile([128, D], BF16, tag=f"wot{mc}", name=f"wot{mc}")
                      for mc in range(4)]
            bt_sb = [mainp.tile([128, S], BF16, tag=f"bt{k}", name=f"bt{k}")
                     for k in range(KT)]

            # ---- Stage A: projections (scoped: xT + W slices + psum) ----
            with (
                tc.tile_pool(name="stageA", bufs=1) as pA,
                tc.tile_pool(name="ppp", bufs=2, space="PSUM") as ppp,
                tc.tile_pool(name="ppv", bufs=2, space="PSUM") as ppv,
            ):
                xt_sb = [pA.tile([128, S], BF16, tag=f"xt{c}", name=f"xt{c}")
                         for c in range(DCH)]
                w_sb = {nm: [pA.tile([128, 512], BF16, tag=f"w{nm}{c}",
                                     name=f"w{nm}{c}") for c in range(DCH)]
                        for nm in ("q", "k", "v")}
                # First-use-order DMA: xt+wq+wk chunks interleaved, then wv.
                for c in range(DCH):
                    nc.sync.dma_start(xt_sb[c][:], d_xt[c * 128:(c + 1) * 128, :])
                    nc.sync.dma_start(w_sb["q"][c][:],
                                      d_wqt[c * 128:(c + 1) * 128, :])
                    nc.sync.dma_start(w_sb["k"][c][:],
                                      d_wkt[c * 128:(c + 1) * 128, :])
                for c in range(DCH):
                    nc.sync.dma_start(w_sb["v"][c][:],
                                      d_wvt[c * 128:(c + 1) * 128, :])

                # Small constants (after the big first-use DMAs).
                ones = constp.tile([1, 512], BF16, tag="ones")
                nc.sync.dma_start(ones[:], d_ones[0:1, :])
                onecol = constp.tile([128, 8], BF16, tag="onecol")
                nc.sync.dma_start(onecol[:], d_ones[:, 0:8])
                bq_sb = constp.tile([128, 4], F32, tag="bq")
                nc.sync.dma_start(bq_sb[:], d_bq[:])
                bk_sb = constp.tile([128, 4], F32, tag="bk")
                nc.sync.dma_start(bk_sb[:], d_bk[:])
                bv_sb = constp.tile([1, 512], BF16, tag="bv")
                nc.sync.dma_start(bv_sb[:], d_bv[:])
                bo_sb = constp.tile([1, D], BF16, tag="bo")
                nc.sync.dma_start(bo_sb[:], d_bo[:])
                # Bias-multiplier tiles (needed from first exp onward).
                for k in range(KT):
                    nc.sync.dma_start(bt_sb[k][:], d_bt[k * 128:(k + 1) * 128, :])
                for mc in range(4):
                    nc.sync.dma_start(wot_sb[mc][:],
                                      d_wot[mc * 128:(mc + 1) * 128, :])

                # Q^T / K^T: psum[d'128, s1024] = sum_c W^T[c][:,d'].T @ xT[c]
                # (st inner so consecutive matmuls share the stationary W).
                for nm, dst, bias in (("q", qt_sb, bq_sb), ("k", kt_sb, bk_sb)):
                    for m in range(4):
                        pp = ppp.tile([128, 1024], F32, tag="pp")
                        for c in range(DCH):
                            for st in range(2):
                                nc.tensor.matmul(
                                    pp[:, st * 512:(st + 1) * 512],
                                    w_sb[nm][c][:, m * 128:(m + 1) * 128],
                                    xt_sb[c][:, st * 512:(st + 1) * 512],
                                    start=(c == 0), stop=(c == DCH - 1))
                        nc.scalar.activation(dst[m][:], pp[:],
                                             IDENT, bias=bias[:, m:m + 1])

                # V natural: psum[s128, dh512] = sum_c xT[c][:,sblk].T @ WvT[c]
                for sb in range(8):
                    ps = ppv.tile([128, 512], F32, tag="pv")
                    for c in range(DCH):
                        nc.tensor.matmul(
                            ps[:],
                            xt_sb[c][:, sb * 128:(sb + 1) * 128],
                            w_sb["v"][c][:],
                            start=(c == 0), stop=False)
                    nc.tensor.matmul(ps[:], ones[0:1, 0:128], bv_sb[:],
                                     start=False, stop=True)
                    vp3 = vp_sb[sb].rearrange("p (h d) -> p h d", d=DH + 1)
                    nc.vector.tensor_copy(
                        vp3[:, :, 0:DH],
                        ps.rearrange("p (h d) -> p h d", d=DH))
                    nc.vector.tensor_copy(
                        vp3[:, :, DH:DH + 1],
                        onecol[:].rearrange("p (h o) -> p h o", o=1))

            # ---- Stage B: attention, software-pipelined by (q-half, pair) --
            with (
                tc.tile_pool(name="ptp", bufs=3) as ptp,
                tc.tile_pool(name="pt2p", bufs=17) as pt2p,
                tc.tile_pool(name="smallp", bufs=2) as smallp,
                tc.tile_pool(name="outp", bufs=2) as outp,
                tc.tile_pool(name="psS", bufs=2, space="PSUM") as psS,
                tc.tile_pool(name="psO", bufs=4, space="PSUM") as psO,
            ):
                def emit_step(cur, prev, pts_prev):
                    """Emit scores for group `cur` interleaved (at k
                    granularity) with the attn@V accumulation for group
                    `prev`, so attnv matmuls fill the PE slots where the
                    scores pipeline would stall on the exp() handshake."""
                    pts = {}
                    pos = []
                    if prev is not None:
                        pm_, pq_ = prev
                        for hh in range(2):
                            pos.append(psO.tile([DH + 1, 512], F32, tag="att",
                                                name=f"po{hh}"))
                    for k in range(KT):
                        if cur is not None:
                            m, q = cur
                            ps = psS.tile([128, 1024], F32, tag="ps")
                            for hh in range(2):
                                nc.tensor.matmul(
                                    ps[:, hh * 512:(hh + 1) * 512],
                                    kt_sb[m][hh * 64:(hh + 1) * 64,
                                             k * 128:(k + 1) * 128],
                                    qt_sb[m][hh * 64:(hh + 1) * 64,
                                             q * 512:(q + 1) * 512],
                                    start=True, stop=True,
                                    tile_position=(hh * 64, 0))
                            pt = ptp.tile([128, 1024], BF16, tag="pt")
                            nc.scalar.activation(pt[:], ps[:], EXP)
                            pt2 = pt2p.tile([128, 1024], BF16, tag="pt2")
                            for hh in range(2):
                                nc.vector.tensor_tensor(
                                    pt2[:, hh * 512:(hh + 1) * 512],
                                    pt[:, hh * 512:(hh + 1) * 512],
                                    bt_sb[k][:, q * 512:(q + 1) * 512],
                                    AluOpType.mult)
                            pts[k] = pt2
                        if prev is not None:
                            for hh in range(2):
                                h = pm_ * 2 + hh
                                nc.tensor.matmul(
                                    pos[hh][:],
                                    vp_sb[k][:, h * (DH + 1):
                                             (h + 1) * (DH + 1)],
                                    pts_prev[k][:, hh * 512:(hh + 1) * 512],
                                    start=(k == 0), stop=(k == KT - 1))
                    if prev is not None:
                        for hh in range(2):
                            zrow = smallp.tile([1, 512], F32, tag="zrow",
                                               name=f"zr{hh}")
                            nc.vector.tensor_copy(zrow[:], pos[hh][DH:DH + 1, :])
                            rec = smallp.tile([1, 512], F32, tag="rec",
                                              name=f"rc{hh}")
                            scr = smallp.tile([1, 512], F32, tag="scr",
                                              name=f"sc{hh}")
                            nc.vector.reciprocal_approx_accurate(rec[:], zrow[:],
                                                                 scr[:])
                            bc = smallp.tile([64, 512], F32, tag="bc",
                                             name=f"bc{hh}")
                            nc.gpsimd.partition_broadcast(bc[:], rec[:])
                            nc.vector.tensor_tensor(
                                at_sb[pm_][hh * 64:(hh + 1) * 64,
                                           pq_ * 512:(pq_ + 1) * 512],
                                pos[hh][0:DH, :], bc[:], AluOpType.mult)
                    return pts

                def emit_outproj(sb_range):
                    # partial output projection for query blocks sb_range
                    # (mc inner-pairs share the stationary at-block).
                    for sb in sb_range:
                        ot = outp.tile([128, D], F32, tag="ot")
                        pc = psS.tile([128, 1024], F32, tag="ps")
                        for mc in range(4):
                            for q2 in range(2):
                                nc.tensor.matmul(
                                    pc[:, q2 * 512:(q2 + 1) * 512],
                                    at_sb[mc][:, sb * 128:(sb + 1) * 128],
                                    wot_sb[mc][:, q2 * 512:(q2 + 1) * 512],
                                    start=(mc == 0), stop=False)
                        for q2 in range(2):
                            nc.tensor.matmul(pc[:, q2 * 512:(q2 + 1) * 512],
                                             ones[0:1, 0:128],
                                             bo_sb[0:1, q2 * 512:(q2 + 1) * 512],
                                             start=False, stop=True)
                            nc.scalar.copy(ot[:, q2 * 512:(q2 + 1) * 512],
                                           pc[:, q2 * 512:(q2 + 1) * 512])
                        nc.sync.dma_start(d_out[sb * 128:(sb + 1) * 128, :],
                                          ot[:])

                groups = [(m, q) for q in range(2) for m in range(4)]
                prev = None
                pts_prev = None
                for g in groups:
                    pts_prev = emit_step(g, prev, pts_prev)
                    if prev == (3, 0):
                        # q-half 0 fully done: overlap its out-projection
                        # with q-half 1 attention.
                        emit_outproj(range(0, 4))
                    prev = g
                emit_step(None, prev, pts_prev)
                emit_outproj(range(4, 8))

    nc.compile()
    return nc


_prog = None


def _get_prog():
    global _prog
    if _prog is None:
        _prog = build_program()
    return _prog


def _host_prep(x, prior_mask, prior_indices, prior_index_mask, u_prev,
               Wq, bq, Wk, bk, Wv, bv, Wo, bo):
    f32 = np.float32
    bf16 = ml_dtypes.bfloat16
    x = np.asarray(x, f32)
    pm = np.asarray(prior_mask, bool)
    idx = np.asarray(prior_indices)
    pim = np.asarray(prior_index_mask, bool)
    u = np.asarray(u_prev, f32).reshape(B)
    Wq, Wk, Wv, Wo = (np.asarray(w, f32) for w in (Wq, Wk, Wv, Wo))
    bq, bk, bv, bo = (np.asarray(v, f32) for v in (bq, bk, bv, bo))

    scale = f32(1.0 / np.sqrt(DH))
    lam = (LAMBDA_MAX * np.exp(-ALPHA * u.astype(np.float64))).astype(f32)
    use_sparse = lam >= SPARSE_THRESHOLD

    # Multiplicative bias, transposed [k, q].
    # Sparse: multiplicity count of key k under prior_indices/mask.
    # Dense: mask ? 1 : exp(-lambda).
    bts_sparse = None
    if use_sparse.any():
        cnt = np.zeros((S, S + 1), np.int32)
        np.add.at(cnt, (np.arange(S)[:, None],
                        np.where(pim, idx, S).astype(np.int64)), 1)
        cnt = cnt[:, :S]
        bts_sparse = np.ascontiguousarray(cnt.T.astype(bf16))

    bts = []
    for b in range(B):
        if use_sparse[b]:
            bts.append(bts_sparse)
        else:
            bd = np.where(pm, f32(1.0), np.exp(f32(-lam[b])))
            bts.append(np.ascontiguousarray(bd.T.astype(bf16)))

    in_maps = []
    for c in range(N_CORES):
        b = c // 2
        hg = c % 2
        hsl = slice(hg * 512, (hg + 1) * 512)
        in_maps.append({
            "xt": np.ascontiguousarray(x[b].T.astype(bf16)),
            "wqt": np.ascontiguousarray((Wq[hsl] * scale).T.astype(bf16)),
            "wkt": np.ascontiguousarray(Wk[hsl].T.astype(bf16)),
            "wvt": np.ascontiguousarray(Wv[hsl].T.astype(bf16)),
            "wot": np.ascontiguousarray(Wo[:, hsl].T.astype(bf16)),
            "bt": bts[b],
            "bq": np.ascontiguousarray((bq[hsl] * scale).reshape(4, 128).T),
            "bk": np.ascontiguousarray(bk[hsl].reshape(4, 128).T),
            "bv": np.ascontiguousarray(bv[hsl].reshape(1, 512).astype(bf16)),
            "bo": np.ascontiguousarray((bo * f32(0.5)).reshape(1, D)
                                       .astype(bf16)),
            "ones": np.ones((128, 512), dtype=bf16),
        })
    return in_maps


def kernel(**inputs):
    in_maps = _host_prep(**inputs)
    nc = _get_prog()
    res = run_bass_kernel_spmd(nc, in_maps, core_ids=list(range(N_CORES)))
    out = np.empty((B, S, D), np.float32)
    for b in range(B):
        out[b] = res.results[2 * b]["out"] + res.results[2 * b + 1]["out"]
    return out


# revision 11
# speedup vs baseline: 1.0309x; 1.0309x over previous
"""Trainium2 Bass kernel for ClippingAttentionEngine.

Sharding: core c -> (batch b = c//2, head-group hg = c%2, 8 heads each).
Each core computes Q/K/V projections for its 8 heads, attention, and the
partial output projection over its head slice; host sums the two per-batch
partials (tensor-parallel over heads, per the sharding hint).

The per-batch sparse/dense branch is folded into a single dense-shaped
program via a MULTIPLICATIVE bias matrix M (bf16, transposed [k,q]):
dense batches get M = mask ? 1 : exp(-lambda); sparse batches get
M = multiplicity count of each key under prior_indices/prior_index_mask
(0 where never indexed). Then softmax(QK^T*scale + log M) == normalize
(exp(QK^T*scale) * M), so P = exp(scores) * M reproduces both branches
exactly. M is applied by the Vector engine (bf16 2x/4x mode) instead of
injecting log M into PSUM with identity matmuls, which frees ~128 tensor
engine instructions per core.

All matmul operands are bf16 (PSUM accumulation stays fp32). Input DMAs
are emitted in first-use order (xt/wq chunks interleaved) so the first
projection matmul can start ~5us in instead of ~27us. The output
projection for the first half of the queries is emitted in the middle of
stage B so its matmuls and DMAs overlap attention of the second half.
"""

import sys

sys.path.insert(0, "/opt/trn_rl_repo")

import ml_dtypes
import numpy as np

import concourse.bass as bass
import concourse.tile as tile
from concourse import bacc, mybir
from concourse.alu_op_type import AluOpType
from concourse.bass_utils import run_bass_kernel_spmd

B, S, D, H = 4, 1024, 1024, 16
DH = D // H          # 64
HPC = 8              # heads per core
N_CORES = 8
KT = S // 128        # 8 k tiles
DCH = D // 128       # 8 contraction chunks
LAMBDA_MAX, ALPHA, SPARSE_THRESHOLD = 10.0, 5.0, 1.0

F32 = mybir.dt.float32
BF16 = mybir.dt.bfloat16
EXP = mybir.ActivationFunctionType.Exp
IDENT = mybir.ActivationFunctionType.Identity


def build_program():
    nc = bacc.Bacc("TRN2", target_bir_lowering=False, debug=False,
                   num_devices=N_CORES)

    d_xt = nc.dram_tensor("xt", [D, S], BF16, kind="ExternalInput").ap()
    d_wqt = nc.dram_tensor("wqt", [D, 512], BF16, kind="ExternalInput").ap()
    d_wkt = nc.dram_tensor("wkt", [D, 512], BF16, kind="ExternalInput").ap()
    d_wvt = nc.dram_tensor("wvt", [D, 512], BF16, kind="ExternalInput").ap()
    d_wot = nc.dram_tensor("wot", [512, D], BF16, kind="ExternalInput").ap()
    d_bt = nc.dram_tensor("bt", [S, S], BF16, kind="ExternalInput").ap()
    d_bq = nc.dram_tensor("bq", [128, 4], F32, kind="ExternalInput").ap()
    d_bk = nc.dram_tensor("bk", [128, 4], F32, kind="ExternalInput").ap()
    d_bv = nc.dram_tensor("bv", [1, 512], BF16, kind="ExternalInput").ap()
    d_bo = nc.dram_tensor("bo", [1, D], BF16, kind="ExternalInput").ap()
    d_ones = nc.dram_tensor("ones", [128, 512], BF16, kind="ExternalInput").ap()
    d_out = nc.dram_tensor("out", [S, D], F32, kind="ExternalOutput").ap()

    with tile.TileContext(nc) as tc:
        with (
            tc.tile_pool(name="const", bufs=1) as constp,
            tc.tile_pool(name="main", bufs=1) as mainp,
        ):
            # Persistent arrays (bf16).
            qt_sb = [mainp.tile([128, S], BF16, tag=f"qt{m}", name=f"qt{m}")
                     for m in range(4)]
            kt_sb = [mainp.tile([128, S], BF16, tag=f"kt{m}", name=f"kt{m}")
                     for m in range(4)]
            vp_sb = [mainp.tile([128, HPC * (DH + 1)], BF16, tag=f"vp{sb}",
                                name=f"vp{sb}") for sb in range(8)]
            at_sb = [mainp.tile([128, S], BF16, tag=f"at{m}", name=f"at{m}")
                     for m in range(4)]
            wot_sb = [mainp.tile([128, D], BF16, tag=f"wot{mc}", name=f"wot{mc}")
                      for mc in range(4)]
            bt_sb = [mainp.tile([128, S], BF16, tag=f"bt{k}", name=f"bt{k}")
                     for k in range(KT)]

            # ---- Stage A: projections (scoped: xT + W slices + psum) ----
            with (
                tc.tile_pool(name="stageA", bufs=1) as pA,
                tc.tile_pool(name="ppp", bufs=2, space="PSUM") as ppp,
                tc.tile_pool(name="ppv", bufs=2, space="PSUM") as ppv,
            ):
                xt_sb = [pA.tile([128, S], BF16, tag=f"xt{c}", name=f"xt{c}")
                         for c in range(DCH)]
                w_sb = {nm: [pA.tile([128, 512], BF16, tag=f"w{nm}{c}",
                                     name=f"w{nm}{c}") for c in range(DCH)]
                        for nm in ("q", "k", "v")}
                # First-use-order DMA: xt+wq+wk chunks interleaved, then wv.
                for c in range(DCH):
                    nc.sync.dma_start(xt_sb[c][:], d_xt[c * 128:(c + 1) * 128, :])
                    nc.sync.dma_start(w_sb["q"][c][:],
                                      d_wqt[c * 128:(c + 1) * 128, :])
                    nc.sync.dma_start(w_sb["k"][c][:],
                                      d_wkt[c * 128:(c + 1) * 128, :])
                for c in range(DCH):
                    nc.sync.dma_start(w_sb["v"][c][:],
                                      d_wvt[c * 128:(c + 1) * 128, :])

                # Small constants (after the big first-use DMAs).
                ones = constp.tile([1, 512], BF16, tag="ones")
                nc.sync.dma_start(ones[:], d_ones[0:1, :])
                onecol = constp.tile([128, 8], BF16, tag="onecol")
                nc.sync.dma_start(onecol[:], d_ones[:, 0:8])
                bq_sb = constp.tile([128, 4], F32, tag="bq")
                nc.sync.dma_start(bq_sb[:], d_bq[:])
                bk_sb = constp.tile([128, 4], F32, tag="bk")
                nc.sync.dma_start(bk_sb[:], d_bk[:])
                bv_sb = constp.tile([1, 512], BF16, tag="bv")
                nc.sync.dma_start(bv_sb[:], d_bv[:])
                bo_sb = constp.tile([1, D], BF16, tag="bo")
                nc.sync.dma_start(bo_sb[:], d_bo[:])
                # Bias-multiplier tiles (needed from first exp onward).
                for k in range(KT):
                    nc.sync.dma_start(bt_sb[k][:], d_bt[k * 128:(k + 1) * 128, :])
                for mc in range(4):
                    nc.sync.dma_start(wot_sb[mc][:],
                                      d_wot[mc * 128:(mc + 1) * 128, :])

                # Q^T / K^T: psum[d'128, s1024] = sum_c W^T[c][:,d'].T @ xT[c]
                # (st inner so consecutive matmuls share the stationary W).
                for nm, dst, bias in (("q", qt_sb, bq_sb), ("k", kt_sb, bk_sb)):
                    for m in range(4):
                        pp = ppp.tile([128, 1024], F32, tag="pp")
                        for c in range(DCH):
                            for st in range(2):
                                nc.tensor.matmul(
                                    pp[:, st * 512:(st + 1) * 512],
                                    w_sb[nm][c][:, m * 128:(m + 1) * 128],
                                    xt_sb[c][:, st * 512:(st + 1) * 512],
                                    start=(c == 0), stop=(c == DCH - 1))
                        nc.scalar.activation(dst[m][:], pp[:],
                                             IDENT, bias=bias[:, m:m + 1])

                # V natural: psum[s128, dh512] = sum_c xT[c][:,sblk].T @ WvT[c]
                for sb in range(8):
                    ps = ppv.tile([128, 512], F32, tag="pv")
                    for c in range(DCH):
                        nc.tensor.matmul(
                            ps[:],
                            xt_sb[c][:, sb * 128:(sb + 1) * 128],
                            w_sb["v"][c][:],
                            start=(c == 0), stop=False)
                    nc.tensor.matmul(ps[:], ones[0:1, 0:128], bv_sb[:],
                                     start=False, stop=True)
                    vp3 = vp_sb[sb].rearrange("p (h d) -> p h d", d=DH + 1)
                    nc.vector.tensor_copy(
                        vp3[:, :, 0:DH],
                        ps.rearrange("p (h d) -> p h d", d=DH))
                    nc.vector.tensor_copy(
                        vp3[:, :, DH:DH + 1],
                        onecol[:].rearrange("p (h o) -> p h o", o=1))

            # ---- Stage B: attention, software-pipelined by (q-half, pair) --
            with (
                tc.tile_pool(name="ptp", bufs=3) as ptp,
                tc.tile_pool(name="pt2p", bufs=17) as pt2p,
                tc.tile_pool(name="smallp", bufs=2) as smallp,
                tc.tile_pool(name="outp", bufs=2) as outp,
                tc.tile_pool(name="psS", bufs=3, space="PSUM") as psS,
                tc.tile_pool(name="psO", bufs=2, space="PSUM") as psO,
            ):
                def emit_step(cur, prev, pts_prev):
                    """Emit scores for group `cur`, then the attn@V
                    accumulation for group `prev`: the attnv matmuls keep
                    the PE busy while the scalar engine finishes the exp()
                    backlog of `cur` (psS bufs=3 lets scores run 3 k-tiles
                    ahead of exp)."""
                    pts = {}
                    pos = []
                    if prev is not None:
                        pm_, pq_ = prev
                        for hh in range(2):
                            pos.append(psO.tile([DH + 1, 512], F32, tag="att",
                                                name=f"po{hh}"))
                    if cur is not None:
                        m, q = cur
                        for k in range(KT):
                            ps = psS.tile([128, 1024], F32, tag="ps")
                            for hh in range(2):
                                nc.tensor.matmul(
                                    ps[:, hh * 512:(hh + 1) * 512],
                                    kt_sb[m][hh * 64:(hh + 1) * 64,
                                             k * 128:(k + 1) * 128],
                                    qt_sb[m][hh * 64:(hh + 1) * 64,
                                             q * 512:(q + 1) * 512],
                                    start=True, stop=True,
                                    tile_position=(hh * 64, 0))
                            pt = ptp.tile([128, 1024], BF16, tag="pt")
                            nc.scalar.activation(pt[:], ps[:], EXP)
                            pt2 = pt2p.tile([128, 1024], BF16, tag="pt2")
                            for hh in range(2):
                                nc.vector.tensor_tensor(
                                    pt2[:, hh * 512:(hh + 1) * 512],
                                    pt[:, hh * 512:(hh + 1) * 512],
                                    bt_sb[k][:, q * 512:(q + 1) * 512],
                                    AluOpType.mult)
                            pts[k] = pt2
                    if prev is not None:
                        for hh in range(2):
                            h = pm_ * 2 + hh
                            for k in range(KT):
                                nc.tensor.matmul(
                                    pos[hh][:],
                                    vp_sb[k][:, h * (DH + 1):
                                             (h + 1) * (DH + 1)],
                                    pts_prev[k][:, hh * 512:(hh + 1) * 512],
                                    start=(k == 0), stop=(k == KT - 1))
                    if prev is not None:
                        for hh in range(2):
                            zrow = smallp.tile([1, 512], F32, tag="zrow",
                                               name=f"zr{hh}")
                            nc.vector.tensor_copy(zrow[:], pos[hh][DH:DH + 1, :])
                            rec = smallp.tile([1, 512], F32, tag="rec",
                                              name=f"rc{hh}")
                            scr = smallp.tile([1, 512], F32, tag="scr",
                                              name=f"sc{hh}")
                            nc.vector.reciprocal_approx_accurate(rec[:], zrow[:],
                                                                 scr[:])
                            bc = smallp.tile([64, 512], F32, tag="bc",
                                             name=f"bc{hh}")
                            nc.gpsimd.partition_broadcast(bc[:], rec[:])
                            nc.vector.tensor_tensor(
                                at_sb[pm_][hh * 64:(hh + 1) * 64,
                                           pq_ * 512:(pq_ + 1) * 512],
                                pos[hh][0:DH, :], bc[:], AluOpType.mult)
                    return pts

                def emit_outproj(sb_range):
                    # partial output projection for query blocks sb_range
                    # (mc inner-pairs share the stationary at-block).
                    for sb in sb_range:
                        ot = outp.tile([128, D], F32, tag="ot")
                        pc = psS.tile([128, 1024], F32, tag="ps")
                        for mc in range(4):
                            for q2 in range(2):
                                nc.tensor.matmul(
                                    pc[:, q2 * 512:(q2 + 1) * 512],
                                    at_sb[mc][:, sb * 128:(sb + 1) * 128],
                                    wot_sb[mc][:, q2 * 512:(q2 + 1) * 512],
                                    start=(mc == 0), stop=False)
                        for q2 in range(2):
                            nc.tensor.matmul(pc[:, q2 * 512:(q2 + 1) * 512],
                                             ones[0:1, 0:128],
                                             bo_sb[0:1, q2 * 512:(q2 + 1) * 512],
                                             start=False, stop=True)
                            nc.scalar.copy(ot[:, q2 * 512:(q2 + 1) * 512],
                                           pc[:, q2 * 512:(q2 + 1) * 512])
                        nc.sync.dma_start(d_out[sb * 128:(sb + 1) * 128, :],
                                          ot[:])

                groups = [(m, q) for q in range(2) for m in range(4)]
                prev = None
                pts_prev = None
                for g in groups:
                    pts_prev = emit_step(g, prev, pts_prev)
                    if prev == (3, 0):
                        # q-half 0 fully done: overlap its out-projection
                        # with q-half 1 attention.
                        emit_outproj(range(0, 4))
                    prev = g
                emit_step(None, prev, pts_prev)
                emit_outproj(range(4, 8))

    nc.compile()
    return nc


_prog = None


def _get_prog():
    global _prog
    if _prog is None:
        _prog = build_program()
    return _prog


def _host_prep(x, prior_mask, prior_indices, prior_index_mask, u_prev,
               Wq, bq, Wk, bk, Wv, bv, Wo, bo):
    f32 = np.float32
    bf16 = ml_dtypes.bfloat16
    x = np.asarray(x, f32)
    pm = np.asarray(prior_mask, bool)
    idx = np.asarray(prior_indices)
    pim = np.asarray(prior_index_mask, bool)
    u = np.asarray(u_prev, f32).reshape(B)
    Wq, Wk, Wv, Wo = (np.asarray(w, f32) for w in (Wq, Wk, Wv, Wo))
    bq, bk, bv, bo = (np.asarray(v, f32) for v in (bq, bk, bv, bo))

    scale = f32(1.0 / np.sqrt(DH))
    lam = (LAMBDA_MAX * np.exp(-ALPHA * u.astype(np.float64))).astype(f32)
    use_sparse = lam >= SPARSE_THRESHOLD

    # Multiplicative bias, transposed [k, q].
    # Sparse: multiplicity count of key k under prior_indices/mask.
    # Dense: mask ? 1 : exp(-lambda).
    bts_sparse = None
    if use_sparse.any():
        cnt = np.zeros((S, S + 1), np.int32)
        np.add.at(cnt, (np.arange(S)[:, None],
                        np.where(pim, idx, S).astype(np.int64)), 1)
        cnt = cnt[:, :S]
        bts_sparse = np.ascontiguousarray(cnt.T.astype(bf16))

    bts = []
    for b in range(B):
        if use_sparse[b]:
            bts.append(bts_sparse)
        else:
            bd = np.where(pm, f32(1.0), np.exp(f32(-lam[b])))
            bts.append(np.ascontiguousarray(bd.T.astype(bf16)))

    in_maps = []
    for c in range(N_CORES):
        b = c // 2
        hg = c % 2
        hsl = slice(hg * 512, (hg + 1) * 512)
        in_maps.append({
            "xt": np.ascontiguousarray(x[b].T.astype(bf16)),
            "wqt": np.ascontiguousarray((Wq[hsl] * scale).T.astype(bf16)),
            "wkt": np.ascontiguousarray(Wk[hsl].T.astype(bf16)),
            "wvt": np.ascontiguousarray(Wv[hsl].T.astype(bf16)),
            "wot": np.ascontiguousarray(Wo[:, hsl].T.astype(bf16)),
            "bt": bts[b],
            "bq": np.ascontiguousarray((bq[hsl] * scale).reshape(4, 128).T),
            "bk": np.ascontiguousarray(bk[hsl].reshape(4, 128).T),
            "bv": np.ascontiguousarray(bv[hsl].reshape(1, 512).astype(bf16)),
            "bo": np.ascontiguousarray((bo * f32(0.5)).reshape(1, D)
                                       .astype(bf16)),
            "ones": np.ones((128, 512), dtype=bf16),
        })
    return in_maps


def kernel(**inputs):
    in_maps = _host_prep(**inputs)
    nc = _get_prog()
    res = run_bass_kernel_spmd(nc, in_maps, core_ids=list(range(N_CORES)))
    out = np.empty((B, S, D), np.float32)
    for b in range(B):
        out[b] = res.results[2 * b]["out"] + res.results[2 * b + 1]["out"]
    return out


# revision 17
# speedup vs baseline: 1.0390x; 1.0079x over previous
"""Trainium2 Bass kernel for ClippingAttentionEngine.

Sharding: core c -> (batch b = c//2, head-group hg = c%2, 8 heads each).
Each core computes Q/K/V projections for its 8 heads, attention, and the
partial output projection over its head slice; host sums the two per-batch
partials (tensor-parallel over heads, per the sharding hint).

The per-batch sparse/dense branch is folded into a single dense-shaped
program via a MULTIPLICATIVE bias matrix M (bf16, transposed [k,q]):
dense batches get M = mask ? 1 : exp(-lambda); sparse batches get
M = multiplicity count of each key under prior_indices/prior_index_mask
(0 where never indexed). Then softmax(QK^T*scale + log M) == normalize
(exp(QK^T*scale) * M), so P = exp(scores) * M reproduces both branches
exactly. M is applied by the Vector engine (bf16 2x/4x mode) instead of
injecting log M into PSUM with identity matmuls, which frees ~128 tensor
engine instructions per core.

All matmul operands are bf16 (PSUM accumulation stays fp32). Input DMAs
are emitted in first-use order (xt/wq chunks interleaved) so the first
projection matmul can start ~5us in instead of ~27us. The output
projection for the first half of the queries is emitted in the middle of
stage B so its matmuls and DMAs overlap attention of the second half.
"""

import sys

sys.path.insert(0, "/opt/trn_rl_repo")

import ml_dtypes
import numpy as np

import concourse.bass as bass
import concourse.tile as tile
from concourse import bacc, mybir
from concourse.alu_op_type import AluOpType
from concourse.bass_utils import run_bass_kernel_spmd

B, S, D, H = 4, 1024, 1024, 16
DH = D // H          # 64
HPC = 8              # heads per core
N_CORES = 8
KT = S // 128        # 8 k tiles
DCH = D // 128       # 8 contraction chunks
LAMBDA_MAX, ALPHA, SPARSE_THRESHOLD = 10.0, 5.0, 1.0

F32 = mybir.dt.float32
BF16 = mybir.dt.bfloat16
FP8 = mybir.dt.float8e4
DR = mybir.MatmulPerfMode.DoubleRow
EXP = mybir.ActivationFunctionType.Exp
IDENT = mybir.ActivationFunctionType.Identity


def build_program():
    nc = bacc.Bacc("TRN2", target_bir_lowering=False, debug=False,
                   num_devices=N_CORES)

    d_xt = nc.dram_tensor("xt", [D, S], BF16, kind="ExternalInput").ap()
    d_wqt = nc.dram_tensor("wqt", [D, 512], BF16, kind="ExternalInput").ap()
    d_wkt = nc.dram_tensor("wkt", [D, 512], BF16, kind="ExternalInput").ap()
    d_wvt = nc.dram_tensor("wvt", [D, 512], BF16, kind="ExternalInput").ap()
    d_wot = nc.dram_tensor("wot", [512, D], BF16, kind="ExternalInput").ap()
    d_bt = nc.dram_tensor("bt", [S, S], BF16, kind="ExternalInput").ap()
    d_bq = nc.dram_tensor("bq", [128, 4], F32, kind="ExternalInput").ap()
    d_bk = nc.dram_tensor("bk", [128, 4], F32, kind="ExternalInput").ap()
    d_bv = nc.dram_tensor("bv", [1, 512], BF16, kind="ExternalInput").ap()
    d_bo = nc.dram_tensor("bo", [1, D], BF16, kind="ExternalInput").ap()
    d_ones = nc.dram_tensor("ones", [128, 512], BF16, kind="ExternalInput").ap()
    d_out = nc.dram_tensor("out", [S, D], F32, kind="ExternalOutput").ap()

    with tile.TileContext(nc) as tc:
        with (
            tc.tile_pool(name="const", bufs=1) as constp,
            tc.tile_pool(name="main", bufs=1) as mainp,
        ):
            # Persistent arrays (bf16).
            qt_sb = [mainp.tile([128, S], BF16, tag=f"qt{m}", name=f"qt{m}")
                     for m in range(4)]
            kt_sb = [mainp.tile([128, S], BF16, tag=f"kt{m}", name=f"kt{m}")
                     for m in range(4)]
            vp_sb = [mainp.tile([128, HPC * (DH + 1)], BF16, tag=f"vp{sb}",
                                name=f"vp{sb}") for sb in range(8)]
            at_sb = [mainp.tile([128, S], BF16, tag=f"at{m}", name=f"at{m}")
                     for m in range(4)]
            wot_sb = [mainp.tile([128, D], BF16, tag=f"wot{mc}", name=f"wot{mc}")
                      for mc in range(4)]
            bt_sb = [mainp.tile([128, S], BF16, tag=f"bt{k}", name=f"bt{k}")
                     for k in range(KT)]

            # ---- Stage A: projections (scoped: xT + W slices + psum) ----
            with (
                tc.tile_pool(name="stageA", bufs=1) as pA,
                tc.tile_pool(name="ppp", bufs=2, space="PSUM") as ppp,
                tc.tile_pool(name="ppv", bufs=2, space="PSUM") as ppv,
            ):
                xt_sb = [pA.tile([128, S], BF16, tag=f"xt{c}", name=f"xt{c}")
                         for c in range(DCH)]
                w_sb = {nm: [pA.tile([128, 512], BF16, tag=f"w{nm}{c}",
                                     name=f"w{nm}{c}") for c in range(DCH)]
                        for nm in ("q", "k", "v")}
                # First-use-order DMA: xt+wq+wk chunks interleaved, with the
                # first chunks on the scalar-engine DGE queue so they don't
                # share packet bandwidth with the bulk on the sync queue.
                for c in range(DCH):
                    eng = nc.scalar if c < 3 else nc.sync
                    eng.dma_start(xt_sb[c][:], d_xt[c * 128:(c + 1) * 128, :])
                    eng.dma_start(w_sb["q"][c][:],
                                  d_wqt[c * 128:(c + 1) * 128, :])
                    eng.dma_start(w_sb["k"][c][:],
                                  d_wkt[c * 128:(c + 1) * 128, :])
                for c in range(DCH):
                    nc.sync.dma_start(w_sb["v"][c][:],
                                      d_wvt[c * 128:(c + 1) * 128, :])

                # Small constants (after the big first-use DMAs).
                ones = constp.tile([1, 512], BF16, tag="ones")
                nc.sync.dma_start(ones[:], d_ones[0:1, :])
                onecol = constp.tile([128, 8], BF16, tag="onecol")
                nc.sync.dma_start(onecol[:], d_ones[:, 0:8])
                bq_sb = constp.tile([128, 4], F32, tag="bq")
                nc.sync.dma_start(bq_sb[:], d_bq[:])
                bk_sb = constp.tile([128, 4], F32, tag="bk")
                nc.sync.dma_start(bk_sb[:], d_bk[:])
                bv_sb = constp.tile([1, 512], BF16, tag="bv")
                nc.sync.dma_start(bv_sb[:], d_bv[:])
                bo_sb = constp.tile([1, D], BF16, tag="bo")
                nc.sync.dma_start(bo_sb[:], d_bo[:])
                # Bias-multiplier tiles (needed from first exp onward).
                for k in range(KT):
                    nc.sync.dma_start(bt_sb[k][:], d_bt[k * 128:(k + 1) * 128, :])
                for mc in range(4):
                    nc.sync.dma_start(wot_sb[mc][:],
                                      d_wot[mc * 128:(mc + 1) * 128, :])

                # Q^T / K^T: psum[d'128, s1024] = sum_c W^T[c][:,d'].T @ xT[c]
                # (st inner so consecutive matmuls share the stationary W).
                for nm, dst, bias in (("q", qt_sb, bq_sb), ("k", kt_sb, bk_sb)):
                    for m in range(4):
                        pp = ppp.tile([128, 1024], F32, tag="pp")
                        for c in range(DCH):
                            for st in range(2):
                                nc.tensor.matmul(
                                    pp[:, st * 512:(st + 1) * 512],
                                    w_sb[nm][c][:, m * 128:(m + 1) * 128],
                                    xt_sb[c][:, st * 512:(st + 1) * 512],
                                    start=(c == 0), stop=(c == DCH - 1))
                        nc.scalar.activation(dst[m][:], pp[:],
                                             IDENT, bias=bias[:, m:m + 1])

                # V natural: psum[s128, dh512] = sum_c xT[c][:,sblk].T @ WvT[c]
                for sb in range(8):
                    ps = ppv.tile([128, 512], F32, tag="pv")
                    for c in range(DCH):
                        nc.tensor.matmul(
                            ps[:],
                            xt_sb[c][:, sb * 128:(sb + 1) * 128],
                            w_sb["v"][c][:],
                            start=(c == 0), stop=False)
                    nc.tensor.matmul(ps[:], ones[0:1, 0:128], bv_sb[:],
                                     start=False, stop=True)
                    vp3 = vp_sb[sb].rearrange("p (h d) -> p h d", d=DH + 1)
                    nc.vector.tensor_copy(
                        vp3[:, :, 0:DH],
                        ps.rearrange("p (h d) -> p h d", d=DH))
                    nc.vector.tensor_copy(
                        vp3[:, :, DH:DH + 1],
                        onecol[:].rearrange("p (h o) -> p h o", o=1))

            # ---- Stage B: attention, software-pipelined by (q-half, pair) --
            with (
                tc.tile_pool(name="ptp", bufs=3) as ptp,
                tc.tile_pool(name="pt2p", bufs=17) as pt2p,
                tc.tile_pool(name="smallp", bufs=2) as smallp,
                tc.tile_pool(name="outp", bufs=2) as outp,
                tc.tile_pool(name="psS", bufs=3, space="PSUM") as psS,
                tc.tile_pool(name="psO", bufs=2, space="PSUM") as psO,
            ):
                def emit_step(cur, prev, pts_prev):
                    """Emit scores for group `cur`, then the attn@V
                    accumulation for group `prev`: the attnv matmuls keep
                    the PE busy while the scalar engine finishes the exp()
                    backlog of `cur` (psS bufs=3 lets scores run 3 k-tiles
                    ahead of exp)."""
                    pts = {}
                    pos = []
                    if prev is not None:
                        pm_, pq_ = prev
                        for hh in range(2):
                            pos.append(psO.tile([DH + 1, 512], F32, tag="att",
                                                name=f"po{hh}"))
                    if cur is not None:
                        m, q = cur
                        for k in range(KT):
                            ps = psS.tile([128, 1024], F32, tag="ps")
                            for hh in range(2):
                                nc.tensor.matmul(
                                    ps[:, hh * 512:(hh + 1) * 512],
                                    kt_sb[m][hh * 64:(hh + 1) * 64,
                                             k * 128:(k + 1) * 128],
                                    qt_sb[m][hh * 64:(hh + 1) * 64,
                                             q * 512:(q + 1) * 512],
                                    start=True, stop=True,
                                    tile_position=(hh * 64, 0))
                            pt = ptp.tile([128, 1024], BF16, tag="pt")
                            nc.scalar.activation(pt[:], ps[:], EXP)
                            pt2 = pt2p.tile([128, 1024], BF16, tag="pt2")
                            for hh in range(2):
                                nc.vector.tensor_tensor(
                                    pt2[:, hh * 512:(hh + 1) * 512],
                                    pt[:, hh * 512:(hh + 1) * 512],
                                    bt_sb[k][:, q * 512:(q + 1) * 512],
                                    AluOpType.mult)
                            pts[k] = pt2
                    if prev is not None:
                        for hh in range(2):
                            h = pm_ * 2 + hh
                            for k in range(KT):
                                nc.tensor.matmul(
                                    pos[hh][:],
                                    vp_sb[k][:, h * (DH + 1):
                                             (h + 1) * (DH + 1)],
                                    pts_prev[k][:, hh * 512:(hh + 1) * 512],
                                    start=(k == 0), stop=(k == KT - 1))
                    if prev is not None:
                        for hh in range(2):
                            zrow = smallp.tile([1, 512], F32, tag="zrow",
                                               name=f"zr{hh}")
                            nc.vector.tensor_copy(zrow[:], pos[hh][DH:DH + 1, :])
                            rec = smallp.tile([1, 512], F32, tag="rec",
                                              name=f"rc{hh}")
                            scr = smallp.tile([1, 512], F32, tag="scr",
                                              name=f"sc{hh}")
                            nc.vector.reciprocal_approx_accurate(rec[:], zrow[:],
                                                                 scr[:])
                            bc = smallp.tile([64, 512], F32, tag="bc",
                                             name=f"bc{hh}")
                            nc.gpsimd.partition_broadcast(bc[:], rec[:])
                            nc.vector.tensor_tensor(
                                at_sb[pm_][hh * 64:(hh + 1) * 64,
                                           pq_ * 512:(pq_ + 1) * 512],
                                pos[hh][0:DH, :], bc[:], AluOpType.mult)
                    return pts

                def emit_outproj(sb_range):
                    # partial output projection for query blocks sb_range
                    # (mc inner-pairs share the stationary at-block).
                    for sb in sb_range:
                        ot = outp.tile([128, D], F32, tag="ot")
                        pc = psS.tile([128, 1024], F32, tag="ps")
                        for mc in range(4):
                            for q2 in range(2):
                                nc.tensor.matmul(
                                    pc[:, q2 * 512:(q2 + 1) * 512],
                                    at_sb[mc][:, sb * 128:(sb + 1) * 128],
                                    wot_sb[mc][:, q2 * 512:(q2 + 1) * 512],
                                    start=(mc == 0), stop=False)
                        for q2 in range(2):
                            nc.tensor.matmul(pc[:, q2 * 512:(q2 + 1) * 512],
                                             ones[0:1, 0:128],
                                             bo_sb[0:1, q2 * 512:(q2 + 1) * 512],
                                             start=False, stop=True)
                            nc.scalar.copy(ot[:, q2 * 512:(q2 + 1) * 512],
                                           pc[:, q2 * 512:(q2 + 1) * 512])
                        nc.sync.dma_start(d_out[sb * 128:(sb + 1) * 128, :],
                                          ot[:])

                groups = [(m, q) for q in range(2) for m in range(4)]
                prev = None
                pts_prev = None
                for g in groups:
                    pts_prev = emit_step(g, prev, pts_prev)
                    if prev == (3, 0):
                        # q-half 0 fully done: overlap its out-projection
                        # with q-half 1 attention.
                        emit_outproj(range(0, 4))
                    prev = g
                emit_step(None, prev, pts_prev)
                emit_outproj(range(4, 8))

    nc.compile()
    return nc


_prog = None


def _get_prog():
    global _prog
    if _prog is None:
        _prog = build_program()
    return _prog


def _host_prep(x, prior_mask, prior_indices, prior_index_mask, u_prev,
               Wq, bq, Wk, bk, Wv, bv, Wo, bo):
    f32 = np.float32
    bf16 = ml_dtypes.bfloat16
    x = np.asarray(x, f32)
    pm = np.asarray(prior_mask, bool)
    idx = np.asarray(prior_indices)
    pim = np.asarray(prior_index_mask, bool)
    u = np.asarray(u_prev, f32).reshape(B)
    Wq, Wk, Wv, Wo = (np.asarray(w, f32) for w in (Wq, Wk, Wv, Wo))
    bq, bk, bv, bo = (np.asarray(v, f32) for v in (bq, bk, bv, bo))

    scale = f32(1.0 / np.sqrt(DH))
    lam = (LAMBDA_MAX * np.exp(-ALPHA * u.astype(np.float64))).astype(f32)
    use_sparse = lam >= SPARSE_THRESHOLD

    # Multiplicative bias, transposed [k, q].
    # Sparse: multiplicity count of key k under prior_indices/mask.
    # Dense: mask ? 1 : exp(-lambda).
    bts_sparse = None
    if use_sparse.any():
        cnt = np.zeros((S, S + 1), np.int32)
        np.add.at(cnt, (np.arange(S)[:, None],
                        np.where(pim, idx, S).astype(np.int64)), 1)
        cnt = cnt[:, :S]
        bts_sparse = np.ascontiguousarray(cnt.T.astype(bf16))

    bts = []
    for b in range(B):
        if use_sparse[b]:
            bts.append(bts_sparse)
        else:
            bd = np.where(pm, f32(1.0), np.exp(f32(-lam[b])))
            bts.append(np.ascontiguousarray(bd.T.astype(bf16)))

    in_maps = []
    for c in range(N_CORES):
        b = c // 2
        hg = c % 2
        hsl = slice(hg * 512, (hg + 1) * 512)
        in_maps.append({
            "xt": np.ascontiguousarray(x[b].T.astype(bf16)),
            "wqt": np.ascontiguousarray((Wq[hsl] * scale).T.astype(bf16)),
            "wkt": np.ascontiguousarray(Wk[hsl].T.astype(bf16)),
            "wvt": np.ascontiguousarray(Wv[hsl].T.astype(bf16)),
            "wot": np.ascontiguousarray(Wo[:, hsl].T.astype(bf16)),
            "bt": bts[b],
            "bq": np.ascontiguousarray((bq[hsl] * scale).reshape(4, 128).T),
            "bk": np.ascontiguousarray(bk[hsl].reshape(4, 128).T),
            "bv": np.ascontiguousarray(bv[hsl].reshape(1, 512).astype(bf16)),
            "bo": np.ascontiguousarray((bo * f32(0.5)).reshape(1, D)
                                       .astype(bf16)),
            "ones": np.ones((128, 512), dtype=bf16),
        })
    return in_maps


def kernel(**inputs):
    in_maps = _host_prep(**inputs)
    nc = _get_prog()
    res = run_bass_kernel_spmd(nc, in_maps, core_ids=list(range(N_CORES)))
    out = np.empty((B, S, D), np.float32)
    for b in range(B):
        out[b] = res.results[2 * b]["out"] + res.results[2 * b + 1]["out"]
    return out


# revision 18
# speedup vs baseline: 1.0409x; 1.0019x over previous
"""Trainium2 Bass kernel for ClippingAttentionEngine.

Sharding: core c -> (batch b = c//2, head-group hg = c%2, 8 heads each).
Each core computes Q/K/V projections for its 8 heads, attention, and the
partial output projection over its head slice; host sums the two per-batch
partials (tensor-parallel over heads, per the sharding hint).

The per-batch sparse/dense branch is folded into a single dense-shaped
program via a MULTIPLICATIVE bias matrix M (bf16, transposed [k,q]):
dense batches get M = mask ? 1 : exp(-lambda); sparse batches get
M = multiplicity count of each key under prior_indices/prior_index_mask
(0 where never indexed). Then softmax(QK^T*scale + log M) == normalize
(exp(QK^T*scale) * M), so P = exp(scores) * M reproduces both branches
exactly. M is applied by the Vector engine (bf16 2x/4x mode) instead of
injecting log M into PSUM with identity matmuls, which frees ~128 tensor
engine instructions per core.

All matmul operands are bf16 (PSUM accumulation stays fp32). Input DMAs
are emitted in first-use order (xt/wq chunks interleaved) so the first
projection matmul can start ~5us in instead of ~27us. The output
projection for the first half of the queries is emitted in the middle of
stage B so its matmuls and DMAs overlap attention of the second half.
"""

import sys

sys.path.insert(0, "/opt/trn_rl_repo")

import ml_dtypes
import numpy as np

import concourse.bass as bass
import concourse.tile as tile
from concourse import bacc, mybir
from concourse.alu_op_type import AluOpType
from concourse.bass_utils import run_bass_kernel_spmd

B, S, D, H = 4, 1024, 1024, 16
DH = D // H          # 64
HPC = 8              # heads per core
N_CORES = 8
KT = S // 128        # 8 k tiles
DCH = D // 128       # 8 contraction chunks
LAMBDA_MAX, ALPHA, SPARSE_THRESHOLD = 10.0, 5.0, 1.0

F32 = mybir.dt.float32
BF16 = mybir.dt.bfloat16
FP8 = mybir.dt.float8e4
DR = mybir.MatmulPerfMode.DoubleRow
EXP = mybir.ActivationFunctionType.Exp
IDENT = mybir.ActivationFunctionType.Identity


def build_program():
    nc = bacc.Bacc("TRN2", target_bir_lowering=False, debug=False,
                   num_devices=N_CORES)

    d_xt = nc.dram_tensor("xt", [D, S], BF16, kind="ExternalInput").ap()
    d_wqt = nc.dram_tensor("wqt", [D, 512], BF16, kind="ExternalInput").ap()
    d_wkt = nc.dram_tensor("wkt", [D, 512], BF16, kind="ExternalInput").ap()
    d_wvt = nc.dram_tensor("wvt", [D, 512], BF16, kind="ExternalInput").ap()
    d_wot = nc.dram_tensor("wot", [512, D], BF16, kind="ExternalInput").ap()
    d_bt = nc.dram_tensor("bt", [S, S], BF16, kind="ExternalInput").ap()
    d_bq = nc.dram_tensor("bq", [128, 4], F32, kind="ExternalInput").ap()
    d_bk = nc.dram_tensor("bk", [128, 4], F32, kind="ExternalInput").ap()
    d_bv = nc.dram_tensor("bv", [1, 512], BF16, kind="ExternalInput").ap()
    d_bo = nc.dram_tensor("bo", [1, D], BF16, kind="ExternalInput").ap()
    d_ones = nc.dram_tensor("ones", [128, 512], BF16, kind="ExternalInput").ap()
    d_out = nc.dram_tensor("out", [S, D], F32, kind="ExternalOutput").ap()

    with tile.TileContext(nc) as tc:
        with (
            tc.tile_pool(name="const", bufs=1) as constp,
            tc.tile_pool(name="main", bufs=1) as mainp,
        ):
            # Persistent arrays (bf16).
            qt_sb = [mainp.tile([128, S], BF16, tag=f"qt{m}", name=f"qt{m}")
                     for m in range(4)]
            kt_sb = [mainp.tile([128, S], BF16, tag=f"kt{m}", name=f"kt{m}")
                     for m in range(4)]
            vp_sb = [mainp.tile([128, HPC * (DH + 1)], BF16, tag=f"vp{sb}",
                                name=f"vp{sb}") for sb in range(8)]
            at_sb = [mainp.tile([128, S], BF16, tag=f"at{m}", name=f"at{m}")
                     for m in range(4)]
            wot_sb = [mainp.tile([128, D], BF16, tag=f"wot{mc}", name=f"wot{mc}")
                      for mc in range(4)]
            bt_sb = [mainp.tile([128, S], BF16, tag=f"bt{k}", name=f"bt{k}")
                     for k in range(KT)]

            # ---- Stage A: projections (scoped: xT + W slices + psum) ----
            with (
                tc.tile_pool(name="stageA", bufs=1) as pA,
                tc.tile_pool(name="ppp", bufs=2, space="PSUM") as ppp,
                tc.tile_pool(name="ppv", bufs=2, space="PSUM") as ppv,
            ):
                xt_sb = [pA.tile([128, S], BF16, tag=f"xt{c}", name=f"xt{c}")
                         for c in range(DCH)]
                w_sb = {nm: [pA.tile([128, 512], BF16, tag=f"w{nm}{c}",
                                     name=f"w{nm}{c}") for c in range(DCH)]
                        for nm in ("q", "k", "v")}
                # First-use-order DMA: xt+wq+wk chunks interleaved, with the
                # first chunks on the scalar-engine DGE queue so they don't
                # share packet bandwidth with the bulk on the sync queue.
                for c in range(DCH):
                    eng = nc.scalar if c < 3 else nc.sync
                    eng.dma_start(xt_sb[c][:], d_xt[c * 128:(c + 1) * 128, :])
                    eng.dma_start(w_sb["q"][c][:],
                                  d_wqt[c * 128:(c + 1) * 128, :])
                    eng.dma_start(w_sb["k"][c][:],
                                  d_wkt[c * 128:(c + 1) * 128, :])
                for c in range(DCH):
                    nc.sync.dma_start(w_sb["v"][c][:],
                                      d_wvt[c * 128:(c + 1) * 128, :])

                # Small constants (after the big first-use DMAs).
                ones = constp.tile([1, 512], BF16, tag="ones")
                nc.sync.dma_start(ones[:], d_ones[0:1, :])
                onecol = constp.tile([128, 8], BF16, tag="onecol")
                nc.sync.dma_start(onecol[:], d_ones[:, 0:8])
                bq_sb = constp.tile([128, 4], F32, tag="bq")
                nc.sync.dma_start(bq_sb[:], d_bq[:])
                bk_sb = constp.tile([128, 4], F32, tag="bk")
                nc.sync.dma_start(bk_sb[:], d_bk[:])
                bv_sb = constp.tile([1, 512], BF16, tag="bv")
                nc.sync.dma_start(bv_sb[:], d_bv[:])
                bo_sb = constp.tile([1, D], BF16, tag="bo")
                nc.sync.dma_start(bo_sb[:], d_bo[:])
                # Bias-multiplier tiles (needed from first exp onward).
                for k in range(KT):
                    nc.sync.dma_start(bt_sb[k][:], d_bt[k * 128:(k + 1) * 128, :])
                for mc in range(4):
                    nc.sync.dma_start(wot_sb[mc][:],
                                      d_wot[mc * 128:(mc + 1) * 128, :])

                # Q^T / K^T: psum[d'128, s1024] = sum_c W^T[c][:,d'].T @ xT[c]
                # (st inner so consecutive matmuls share the stationary W).
                for nm, dst, bias in (("q", qt_sb, bq_sb), ("k", kt_sb, bk_sb)):
                    for m in range(4):
                        pp = ppp.tile([128, 1024], F32, tag="pp")
                        for c in range(DCH):
                            for st in range(2):
                                nc.tensor.matmul(
                                    pp[:, st * 512:(st + 1) * 512],
                                    w_sb[nm][c][:, m * 128:(m + 1) * 128],
                                    xt_sb[c][:, st * 512:(st + 1) * 512],
                                    start=(c == 0), stop=(c == DCH - 1))
                        nc.scalar.activation(dst[m][:], pp[:],
                                             IDENT, bias=bias[:, m:m + 1])

                # V natural: psum[s128, dh512] = sum_c xT[c][:,sblk].T @ WvT[c]
                for sb in range(8):
                    ps = ppv.tile([128, 512], F32, tag="pv")
                    for c in range(DCH):
                        nc.tensor.matmul(
                            ps[:],
                            xt_sb[c][:, sb * 128:(sb + 1) * 128],
                            w_sb["v"][c][:],
                            start=(c == 0), stop=False)
                    nc.tensor.matmul(ps[:], ones[0:1, 0:128], bv_sb[:],
                                     start=False, stop=True)
                    vp3 = vp_sb[sb].rearrange("p (h d) -> p h d", d=DH + 1)
                    nc.vector.tensor_copy(
                        vp3[:, :, 0:DH],
                        ps.rearrange("p (h d) -> p h d", d=DH))
                    nc.vector.tensor_copy(
                        vp3[:, :, DH:DH + 1],
                        onecol[:].rearrange("p (h o) -> p h o", o=1))

            # ---- Stage B: attention, software-pipelined by (q-half, pair) --
            with (
                tc.tile_pool(name="ptp", bufs=3) as ptp,
                tc.tile_pool(name="pt2p", bufs=17) as pt2p,
                tc.tile_pool(name="smallp", bufs=2) as smallp,
                tc.tile_pool(name="outp", bufs=2) as outp,
                tc.tile_pool(name="psS", bufs=3, space="PSUM") as psS,
                tc.tile_pool(name="psO", bufs=2, space="PSUM") as psO,
            ):
                def emit_step(cur, prev, pts_prev):
                    """Emit scores for group `cur`, then the attn@V
                    accumulation for group `prev`: the attnv matmuls keep
                    the PE busy while the scalar engine finishes the exp()
                    backlog of `cur` (psS bufs=3 lets scores run 3 k-tiles
                    ahead of exp)."""
                    pts = {}
                    pos = []
                    if prev is not None:
                        pm_, pq_ = prev
                        for hh in range(2):
                            pos.append(psO.tile([DH + 1, 512], F32, tag="att",
                                                name=f"po{hh}"))
                    if cur is not None:
                        m, q = cur
                        for k in range(KT):
                            ps = psS.tile([128, 1024], F32, tag="ps")
                            for hh in range(2):
                                nc.tensor.matmul(
                                    ps[:, hh * 512:(hh + 1) * 512],
                                    kt_sb[m][hh * 64:(hh + 1) * 64,
                                             k * 128:(k + 1) * 128],
                                    qt_sb[m][hh * 64:(hh + 1) * 64,
                                             q * 512:(q + 1) * 512],
                                    start=True, stop=True,
                                    tile_position=(hh * 64, 0))
                            pt = ptp.tile([128, 1024], BF16, tag="pt")
                            nc.scalar.activation(pt[:], ps[:], EXP)
                            pt2 = pt2p.tile([128, 1024], BF16, tag="pt2")
                            for hh in range(2):
                                nc.vector.tensor_tensor(
                                    pt2[:, hh * 512:(hh + 1) * 512],
                                    pt[:, hh * 512:(hh + 1) * 512],
                                    bt_sb[k][:, q * 512:(q + 1) * 512],
                                    AluOpType.mult)
                            pts[k] = pt2
                    if prev is not None:
                        for hh in range(2):
                            h = pm_ * 2 + hh
                            for k in range(KT):
                                nc.tensor.matmul(
                                    pos[hh][:],
                                    vp_sb[k][:, h * (DH + 1):
                                             (h + 1) * (DH + 1)],
                                    pts_prev[k][:, hh * 512:(hh + 1) * 512],
                                    start=(k == 0), stop=(k == KT - 1))
                    if prev is not None:
                        for hh in range(2):
                            zrow = smallp.tile([1, 512], F32, tag="zrow",
                                               name=f"zr{hh}")
                            nc.vector.tensor_copy(zrow[:], pos[hh][DH:DH + 1, :])
                            rec = smallp.tile([1, 512], F32, tag="rec",
                                              name=f"rc{hh}")
                            scr = smallp.tile([1, 512], F32, tag="scr",
                                              name=f"sc{hh}")
                            nc.vector.reciprocal_approx_accurate(rec[:], zrow[:],
                                                                 scr[:])
                            bc = smallp.tile([64, 512], F32, tag="bc",
                                             name=f"bc{hh}")
                            nc.gpsimd.partition_broadcast(bc[:], rec[:])
                            nc.vector.tensor_tensor(
                                at_sb[pm_][hh * 64:(hh + 1) * 64,
                                           pq_ * 512:(pq_ + 1) * 512],
                                pos[hh][0:DH, :], bc[:], AluOpType.mult)
                    return pts

                def emit_outproj(sb_range):
                    # partial output projection for query blocks sb_range
                    # (mc inner-pairs share the stationary at-block).
                    for sb in sb_range:
                        ot = outp.tile([128, D], F32, tag="ot")
                        pc = psS.tile([128, 1024], F32, tag="ps")
                        for mc in range(4):
                            for q2 in range(2):
                                nc.tensor.matmul(
                                    pc[:, q2 * 512:(q2 + 1) * 512],
                                    at_sb[mc][:, sb * 128:(sb + 1) * 128],
                                    wot_sb[mc][:, q2 * 512:(q2 + 1) * 512],
                                    start=(mc == 0), stop=False)
                        for q2 in range(2):
                            nc.tensor.matmul(pc[:, q2 * 512:(q2 + 1) * 512],
                                             ones[0:1, 0:128],
                                             bo_sb[0:1, q2 * 512:(q2 + 1) * 512],
                                             start=False, stop=True)
                            # Copy on DVE, not ACT: keeps the scalar engine's
                            # exp cadence (which paces the scores pipeline)
                            # free of interruptions.
                            nc.vector.tensor_copy(
                                ot[:, q2 * 512:(q2 + 1) * 512],
                                pc[:, q2 * 512:(q2 + 1) * 512])
                            nc.sync.dma_start(
                                d_out[sb * 128:(sb + 1) * 128,
                                      q2 * 512:(q2 + 1) * 512],
                                ot[:, q2 * 512:(q2 + 1) * 512])

                groups = [(m, q) for q in range(2) for m in range(4)]
                prev = None
                pts_prev = None
                for g in groups:
                    pts_prev = emit_step(g, prev, pts_prev)
                    if prev == (3, 0):
                        # q-half 0 fully done: overlap its out-projection
                        # with q-half 1 attention.
                        emit_outproj(range(0, 4))
                    prev = g
                emit_step(None, prev, pts_prev)
                emit_outproj(range(4, 8))

    nc.compile()
    return nc


_prog = None


def _get_prog():
    global _prog
    if _prog is None:
        _prog = build_program()
    return _prog


def _host_prep(x, prior_mask, prior_indices, prior_index_mask, u_prev,
               Wq, bq, Wk, bk, Wv, bv, Wo, bo):
    f32 = np.float32
    bf16 = ml_dtypes.bfloat16
    x = np.asarray(x, f32)
    pm = np.asarray(prior_mask, bool)
    idx = np.asarray(prior_indices)
    pim = np.asarray(prior_index_mask, bool)
    u = np.asarray(u_prev, f32).reshape(B)
    Wq, Wk, Wv, Wo = (np.asarray(w, f32) for w in (Wq, Wk, Wv, Wo))
    bq, bk, bv, bo = (np.asarray(v, f32) for v in (bq, bk, bv, bo))

    scale = f32(1.0 / np.sqrt(DH))
    lam = (LAMBDA_MAX * np.exp(-ALPHA * u.astype(np.float64))).astype(f32)
    use_sparse = lam >= SPARSE_THRESHOLD

    # Multiplicative bias, transposed [k, q].
    # Sparse: multiplicity count of key k under prior_indices/mask.
    # Dense: mask ? 1 : exp(-lambda).
    bts_sparse = None
    if use_sparse.any():
        cnt = np.zeros((S, S + 1), np.int32)
        np.add.at(cnt, (np.arange(S)[:, None],
                        np.where(pim, idx, S).astype(np.int64)), 1)
        cnt = cnt[:, :S]
        bts_sparse = np.ascontiguousarray(cnt.T.astype(bf16))

    bts = []
    for b in range(B):
        if use_sparse[b]:
            bts.append(bts_sparse)
        else:
            bd = np.where(pm, f32(1.0), np.exp(f32(-lam[b])))
            bts.append(np.ascontiguousarray(bd.T.astype(bf16)))

    in_maps = []
    for c in range(N_CORES):
        b = c // 2
        hg = c % 2
        hsl = slice(hg * 512, (hg + 1) * 512)
        in_maps.append({
            "xt": np.ascontiguousarray(x[b].T.astype(bf16)),
            "wqt": np.ascontiguousarray((Wq[hsl] * scale).T.astype(bf16)),
            "wkt": np.ascontiguousarray(Wk[hsl].T.astype(bf16)),
            "wvt": np.ascontiguousarray(Wv[hsl].T.astype(bf16)),
            "wot": np.ascontiguousarray(Wo[:, hsl].T.astype(bf16)),
            "bt": bts[b],
            "bq": np.ascontiguousarray((bq[hsl] * scale).reshape(4, 128).T),
            "bk": np.ascontiguousarray(bk[hsl].reshape(4, 128).T),
            "bv": np.ascontiguousarray(bv[hsl].reshape(1, 512).astype(bf16)),
            "bo": np.ascontiguousarray((bo * f32(0.5)).reshape(1, D)
                                       .astype(bf16)),
            "ones": np.ones((128, 512), dtype=bf16),
        })
    return in_maps


def kernel(**inputs):
    in_maps = _host_prep(**inputs)
    nc = _get_prog()
    res = run_bass_kernel_spmd(nc, in_maps, core_ids=list(range(N_CORES)))
    out = np.empty((B, S, D), np.float32)
    for b in range(B):
        out[b] = res.results[2 * b]["out"] + res.results[2 * b + 1]["out"]
    return out


# revision 20
# speedup vs baseline: 1.0541x; 1.0127x over previous
"""Trainium2 Bass kernel for ClippingAttentionEngine.

Sharding: core c -> (batch b = c//2, head-group hg = c%2, 8 heads each).
Each core computes Q/K/V projections for its 8 heads, attention, and the
partial output projection over its head slice; host sums the two per-batch
partials (tensor-parallel over heads, per the sharding hint).

The per-batch sparse/dense branch is folded into a single dense-shaped
program via a MULTIPLICATIVE bias matrix M (bf16, transposed [k,q]):
dense batches get M = mask ? 1 : exp(-lambda); sparse batches get
M = multiplicity count of each key under prior_indices/prior_index_mask
(0 where never indexed). Then softmax(QK^T*scale + log M) == normalize
(exp(QK^T*scale) * M), so P = exp(scores) * M reproduces both branches
exactly. M is applied by the Vector engine (bf16 2x/4x mode) instead of
injecting log M into PSUM with identity matmuls, which frees ~128 tensor
engine instructions per core.

All matmul operands are bf16 (PSUM accumulation stays fp32). Input DMAs
are emitted in first-use order (xt/wq chunks interleaved) so the first
projection matmul can start ~5us in instead of ~27us. The output
projection for the first half of the queries is emitted in the middle of
stage B so its matmuls and DMAs overlap attention of the second half.
"""

import sys

sys.path.insert(0, "/opt/trn_rl_repo")

import ml_dtypes
import numpy as np

import concourse.bass as bass
import concourse.tile as tile
from concourse import bacc, mybir
from concourse.alu_op_type import AluOpType
from concourse.bass_utils import run_bass_kernel_spmd

B, S, D, H = 4, 1024, 1024, 16
DH = D // H          # 64
HPC = 8              # heads per core
N_CORES = 8
KT = S // 128        # 8 k tiles
DCH = D // 128       # 8 contraction chunks
LAMBDA_MAX, ALPHA, SPARSE_THRESHOLD = 10.0, 5.0, 1.0

F32 = mybir.dt.float32
BF16 = mybir.dt.bfloat16
FP8 = mybir.dt.float8e4
DR = mybir.MatmulPerfMode.DoubleRow
EXP = mybir.ActivationFunctionType.Exp
IDENT = mybir.ActivationFunctionType.Identity


def build_program():
    nc = bacc.Bacc("TRN2", target_bir_lowering=False, debug=False,
                   num_devices=N_CORES)

    # x^T, Wq^T, Wk^T, Wv^T packed column-wise so one DMA per 128-row chunk
    # brings everything the projections need (8 dispatches instead of 24).
    d_xtw = nc.dram_tensor("xtw", [D, S + 3 * 512], BF16,
                           kind="ExternalInput").ap()
    d_wot = nc.dram_tensor("wot", [512, D], BF16, kind="ExternalInput").ap()
    d_bt = nc.dram_tensor("bt", [S, S], BF16, kind="ExternalInput").ap()
    d_bq = nc.dram_tensor("bq", [128, 4], F32, kind="ExternalInput").ap()
    d_bk = nc.dram_tensor("bk", [128, 4], F32, kind="ExternalInput").ap()
    d_bv = nc.dram_tensor("bv", [1, 512], BF16, kind="ExternalInput").ap()
    d_bo = nc.dram_tensor("bo", [1, D], BF16, kind="ExternalInput").ap()
    d_ones = nc.dram_tensor("ones", [128, 512], BF16, kind="ExternalInput").ap()
    d_out = nc.dram_tensor("out", [S, D], F32, kind="ExternalOutput").ap()

    with tile.TileContext(nc) as tc:
        with (
            tc.tile_pool(name="const", bufs=1) as constp,
            tc.tile_pool(name="main", bufs=1) as mainp,
        ):
            # Persistent arrays (bf16).
            qt_sb = [mainp.tile([128, S], BF16, tag=f"qt{m}", name=f"qt{m}")
                     for m in range(4)]
            kt_sb = [mainp.tile([128, S], BF16, tag=f"kt{m}", name=f"kt{m}")
                     for m in range(4)]
            vp_sb = [mainp.tile([128, HPC * (DH + 1)], BF16, tag=f"vp{sb}",
                                name=f"vp{sb}") for sb in range(8)]
            at_sb = [mainp.tile([128, S], BF16, tag=f"at{m}", name=f"at{m}")
                     for m in range(4)]
            wot_sb = [mainp.tile([128, D], BF16, tag=f"wot{mc}", name=f"wot{mc}")
                      for mc in range(4)]
            bt_sb = [mainp.tile([128, S], BF16, tag=f"bt{k}", name=f"bt{k}")
                     for k in range(KT)]

            # ---- Stage A: projections (scoped: xT + W slices + psum) ----
            with (
                tc.tile_pool(name="stageA", bufs=1) as pA,
                tc.tile_pool(name="ppp", bufs=2, space="PSUM") as ppp,
                tc.tile_pool(name="ppv", bufs=2, space="PSUM") as ppv,
            ):
                W_ = S + 3 * 512
                xtw_sb = [pA.tile([128, W_], BF16, tag=f"xtw{c}",
                                  name=f"xtw{c}") for c in range(DCH)]
                # One DMA per chunk; first chunks on the scalar-engine DGE
                # queue so they don't share packet bandwidth with the bulk.
                for c in range(DCH):
                    eng = nc.scalar if c < 3 else nc.sync
                    eng.dma_start(xtw_sb[c][:], d_xtw[c * 128:(c + 1) * 128, :])
                xt_sb = [t[:, 0:S] for t in xtw_sb]
                w_sb = {nm: [xtw_sb[c][:, S + i * 512:S + (i + 1) * 512]
                             for c in range(DCH)]
                        for i, nm in enumerate(("q", "k", "v"))}

                # Small constants (after the big first-use DMAs).
                ones = constp.tile([1, 512], BF16, tag="ones")
                nc.sync.dma_start(ones[:], d_ones[0:1, :])
                onecol = constp.tile([128, 8], BF16, tag="onecol")
                nc.sync.dma_start(onecol[:], d_ones[:, 0:8])
                bq_sb = constp.tile([128, 4], F32, tag="bq")
                nc.sync.dma_start(bq_sb[:], d_bq[:])
                bk_sb = constp.tile([128, 4], F32, tag="bk")
                nc.sync.dma_start(bk_sb[:], d_bk[:])
                bv_sb = constp.tile([1, 512], BF16, tag="bv")
                nc.sync.dma_start(bv_sb[:], d_bv[:])
                bo_sb = constp.tile([1, D], BF16, tag="bo")
                nc.sync.dma_start(bo_sb[:], d_bo[:])
                # Bias-multiplier tiles (needed from first exp onward).
                for k in range(KT):
                    nc.sync.dma_start(bt_sb[k][:], d_bt[k * 128:(k + 1) * 128, :])
                for mc in range(4):
                    nc.sync.dma_start(wot_sb[mc][:],
                                      d_wot[mc * 128:(mc + 1) * 128, :])

                # Q^T / K^T: psum[d'128, s1024] = sum_c W^T[c][:,d'].T @ xT[c]
                # (st inner so consecutive matmuls share the stationary W).
                for nm, dst, bias in (("q", qt_sb, bq_sb), ("k", kt_sb, bk_sb)):
                    for m in range(4):
                        pp = ppp.tile([128, 1024], F32, tag="pp")
                        for c in range(DCH):
                            for st in range(2):
                                nc.tensor.matmul(
                                    pp[:, st * 512:(st + 1) * 512],
                                    w_sb[nm][c][:, m * 128:(m + 1) * 128],
                                    xt_sb[c][:, st * 512:(st + 1) * 512],
                                    start=(c == 0), stop=(c == DCH - 1))
                        nc.scalar.activation(dst[m][:], pp[:],
                                             IDENT, bias=bias[:, m:m + 1])

                # V natural: psum[s128, dh512] = sum_c xT[c][:,sblk].T @ WvT[c]
                for sb in range(8):
                    ps = ppv.tile([128, 512], F32, tag="pv")
                    for c in range(DCH):
                        nc.tensor.matmul(
                            ps[:],
                            xt_sb[c][:, sb * 128:(sb + 1) * 128],
                            w_sb["v"][c][:],
                            start=(c == 0), stop=False)
                    nc.tensor.matmul(ps[:], ones[0:1, 0:128], bv_sb[:],
                                     start=False, stop=True)
                    vp3 = vp_sb[sb].rearrange("p (h d) -> p h d", d=DH + 1)
                    nc.vector.tensor_copy(
                        vp3[:, :, 0:DH],
                        ps.rearrange("p (h d) -> p h d", d=DH))
                    nc.vector.tensor_copy(
                        vp3[:, :, DH:DH + 1],
                        onecol[:].rearrange("p (h o) -> p h o", o=1))

            # ---- Stage B: attention, software-pipelined by (q-half, pair) --
            with (
                tc.tile_pool(name="ptp", bufs=3) as ptp,
                tc.tile_pool(name="pt2p", bufs=17) as pt2p,
                tc.tile_pool(name="smallp", bufs=2) as smallp,
                tc.tile_pool(name="outp", bufs=2) as outp,
                tc.tile_pool(name="psS", bufs=3, space="PSUM") as psS,
                tc.tile_pool(name="psO", bufs=2, space="PSUM") as psO,
            ):
                def emit_step(cur, prev, pts_prev):
                    """Emit scores for group `cur`, then the attn@V
                    accumulation for group `prev`: the attnv matmuls keep
                    the PE busy while the scalar engine finishes the exp()
                    backlog of `cur` (psS bufs=3 lets scores run 3 k-tiles
                    ahead of exp)."""
                    pts = {}
                    pos = []
                    if prev is not None:
                        pm_, pq_ = prev
                        for hh in range(2):
                            pos.append(psO.tile([DH + 1, 512], F32, tag="att",
                                                name=f"po{hh}"))
                    if cur is not None:
                        m, q = cur
                        for k in range(KT):
                            ps = psS.tile([128, 1024], F32, tag="ps")
                            for hh in range(2):
                                nc.tensor.matmul(
                                    ps[:, hh * 512:(hh + 1) * 512],
                                    kt_sb[m][hh * 64:(hh + 1) * 64,
                                             k * 128:(k + 1) * 128],
                                    qt_sb[m][hh * 64:(hh + 1) * 64,
                                             q * 512:(q + 1) * 512],
                                    start=True, stop=True,
                                    tile_position=(hh * 64, 0))
                            pt = ptp.tile([128, 1024], BF16, tag="pt")
                            nc.scalar.activation(pt[:], ps[:], EXP)
                            pt2 = pt2p.tile([128, 1024], BF16, tag="pt2")
                            for hh in range(2):
                                nc.vector.tensor_tensor(
                                    pt2[:, hh * 512:(hh + 1) * 512],
                                    pt[:, hh * 512:(hh + 1) * 512],
                                    bt_sb[k][:, q * 512:(q + 1) * 512],
                                    AluOpType.mult)
                            pts[k] = pt2
                    if prev is not None:
                        for hh in range(2):
                            h = pm_ * 2 + hh
                            for k in range(KT):
                                nc.tensor.matmul(
                                    pos[hh][:],
                                    vp_sb[k][:, h * (DH + 1):
                                             (h + 1) * (DH + 1)],
                                    pts_prev[k][:, hh * 512:(hh + 1) * 512],
                                    start=(k == 0), stop=(k == KT - 1))
                    if prev is not None:
                        for hh in range(2):
                            zrow = smallp.tile([1, 512], F32, tag="zrow",
                                               name=f"zr{hh}")
                            nc.vector.tensor_copy(zrow[:], pos[hh][DH:DH + 1, :])
                            rec = smallp.tile([1, 512], F32, tag="rec",
                                              name=f"rc{hh}")
                            scr = smallp.tile([1, 512], F32, tag="scr",
                                              name=f"sc{hh}")
                            nc.vector.reciprocal_approx_accurate(rec[:], zrow[:],
                                                                 scr[:])
                            bc = smallp.tile([64, 512], F32, tag="bc",
                                             name=f"bc{hh}")
                            nc.gpsimd.partition_broadcast(bc[:], rec[:])
                            nc.vector.tensor_tensor(
                                at_sb[pm_][hh * 64:(hh + 1) * 64,
                                           pq_ * 512:(pq_ + 1) * 512],
                                pos[hh][0:DH, :], bc[:], AluOpType.mult)
                    return pts

                def emit_outproj(sb_range):
                    # partial output projection for query blocks sb_range
                    # (mc inner-pairs share the stationary at-block).
                    for sb in sb_range:
                        ot = outp.tile([128, D], F32, tag="ot")
                        pc = psS.tile([128, 1024], F32, tag="ps")
                        for mc in range(4):
                            for q2 in range(2):
                                nc.tensor.matmul(
                                    pc[:, q2 * 512:(q2 + 1) * 512],
                                    at_sb[mc][:, sb * 128:(sb + 1) * 128],
                                    wot_sb[mc][:, q2 * 512:(q2 + 1) * 512],
                                    start=(mc == 0), stop=False)
                        for q2 in range(2):
                            nc.tensor.matmul(pc[:, q2 * 512:(q2 + 1) * 512],
                                             ones[0:1, 0:128],
                                             bo_sb[0:1, q2 * 512:(q2 + 1) * 512],
                                             start=False, stop=True)
                            # Copy on DVE, not ACT: keeps the scalar engine's
                            # exp cadence (which paces the scores pipeline)
                            # free of interruptions.
                            nc.vector.tensor_copy(
                                ot[:, q2 * 512:(q2 + 1) * 512],
                                pc[:, q2 * 512:(q2 + 1) * 512])
                            nc.sync.dma_start(
                                d_out[sb * 128:(sb + 1) * 128,
                                      q2 * 512:(q2 + 1) * 512],
                                ot[:, q2 * 512:(q2 + 1) * 512])

                groups = [(m, q) for q in range(2) for m in range(4)]
                prev = None
                pts_prev = None
                for g in groups:
                    pts_prev = emit_step(g, prev, pts_prev)
                    if prev == (3, 0):
                        # q-half 0 fully done: overlap its out-projection
                        # with q-half 1 attention.
                        emit_outproj(range(0, 4))
                    prev = g
                emit_step(None, prev, pts_prev)
                emit_outproj(range(4, 8))

    nc.compile()
    return nc


_prog = None


def _get_prog():
    global _prog
    if _prog is None:
        _prog = build_program()
    return _prog


def _host_prep(x, prior_mask, prior_indices, prior_index_mask, u_prev,
               Wq, bq, Wk, bk, Wv, bv, Wo, bo):
    f32 = np.float32
    bf16 = ml_dtypes.bfloat16
    x = np.asarray(x, f32)
    pm = np.asarray(prior_mask, bool)
    idx = np.asarray(prior_indices)
    pim = np.asarray(prior_index_mask, bool)
    u = np.asarray(u_prev, f32).reshape(B)
    Wq, Wk, Wv, Wo = (np.asarray(w, f32) for w in (Wq, Wk, Wv, Wo))
    bq, bk, bv, bo = (np.asarray(v, f32) for v in (bq, bk, bv, bo))

    scale = f32(1.0 / np.sqrt(DH))
    lam = (LAMBDA_MAX * np.exp(-ALPHA * u.astype(np.float64))).astype(f32)
    use_sparse = lam >= SPARSE_THRESHOLD

    # Multiplicative bias, transposed [k, q].
    # Sparse: multiplicity count of key k under prior_indices/mask.
    # Dense: mask ? 1 : exp(-lambda).
    bts_sparse = None
    if use_sparse.any():
        cnt = np.zeros((S, S + 1), np.int32)
        np.add.at(cnt, (np.arange(S)[:, None],
                        np.where(pim, idx, S).astype(np.int64)), 1)
        cnt = cnt[:, :S]
        bts_sparse = np.ascontiguousarray(cnt.T.astype(bf16))

    bts = []
    for b in range(B):
        if use_sparse[b]:
            bts.append(bts_sparse)
        else:
            bd = np.where(pm, f32(1.0), np.exp(f32(-lam[b])))
            bts.append(np.ascontiguousarray(bd.T.astype(bf16)))

    in_maps = []
    for c in range(N_CORES):
        b = c // 2
        hg = c % 2
        hsl = slice(hg * 512, (hg + 1) * 512)
        in_maps.append({
            "xtw": np.ascontiguousarray(np.concatenate(
                [x[b].T, (Wq[hsl] * scale).T, Wk[hsl].T, Wv[hsl].T],
                axis=1).astype(bf16)),
            "wot": np.ascontiguousarray(Wo[:, hsl].T.astype(bf16)),
            "bt": bts[b],
            "bq": np.ascontiguousarray((bq[hsl] * scale).reshape(4, 128).T),
            "bk": np.ascontiguousarray(bk[hsl].reshape(4, 128).T),
            "bv": np.ascontiguousarray(bv[hsl].reshape(1, 512).astype(bf16)),
            "bo": np.ascontiguousarray((bo * f32(0.5)).reshape(1, D)
                                       .astype(bf16)),
            "ones": np.ones((128, 512), dtype=bf16),
        })
    return in_maps


def kernel(**inputs):
    in_maps = _host_prep(**inputs)
    nc = _get_prog()
    res = run_bass_kernel_spmd(nc, in_maps, core_ids=list(range(N_CORES)))
    out = np.empty((B, S, D), np.float32)
    for b in range(B):
        out[b] = res.results[2 * b]["out"] + res.results[2 * b + 1]["out"]
    return out
